# revision 1
# baseline (speedup 1.0000x reference)
"""MoE layer (top-2 of 8 experts), expert-parallel across 8 Trainium2 NeuronCores.

Strategy (self-contained; shapes hardcoded for B=4,T=1024,D=1024,E=8,K=2,H=4096):
  - Host: gate logits + top-2 + softmax, dispatch tokens per expert (capacity C
    = max expert load, chosen at runtime), combine weighted expert outputs.
    (The gate is a 4096x1024x8 matmul - noise next to the expert FFNs; the
    routing/compaction is control-flow-heavy and is done host-side as part of
    the shard/unshard step.)
  - Device, SPMD over 8 cores (core e owns expert e): transposed-layout FFN
      hT = gelu(w1.T @ xT + b1)   [H, C]
      yT = w2.T @ hT + b2         [D, C]
    bf16 matmul inputs, f32 PSUM accumulation.  Both weights are already
    [K, M] ("lhsT") in natural storage order and tokens are fed transposed
    [D, C], so no device-side transposes are needed.
"""

import os

import numpy as np
import ml_dtypes

B, T, D = 4, 1024, 1024
E, K, H = 8, 2, 4 * 1024
N = B * T
P = 128
KD = D // P           # 8  k-tiles in GEMM1 / output d-tiles in GEMM2
MH = H // P           # 32 m-tiles in GEMM1 / k-tiles in GEMM2
BF16 = ml_dtypes.bfloat16

LAST_EXEC_TIME_NS = None
_cached_nc = {}


def _chunks(c):
    """Split the token dim into PSUM-bank-sized (<=512 f32) slices.

    The first slice is maxed at 512 so the first psum group buys the DMA ring
    more time to deliver the later x-slices; the rest split near-equally but
    stay >=256 so LDWEIGHTS (~107ns) keeps hiding under the matmul stream.
    """
    n = -(-c // 512)
    base = c // n
    rem = c - base * n
    sizes = [base + (1 if i < rem else 0) for i in range(n)]
    out, off = [], 0
    for s in sizes:
        out.append(slice(off, off + s))
        off += s
    return out


def _ensure_ntff_hook():
    """Register the axon NTFF profile hook if the image lacks antenv.axon_hooks."""
    import sys
    import types
    try:
        from antenv.axon_hooks import get_axon_ntff_profile_hook
        return get_axon_ntff_profile_hook() is not None
    except ImportError:
        pass
    try:
        import antenv
        from trn_agent_boot.trn_boot import _ntff_profile_via_ctypes
        mod = types.ModuleType("antenv.axon_hooks")
        holder = [None]
        mod.set_axon_ntff_profile_hook = lambda h: holder.__setitem__(0, h)
        mod.get_axon_ntff_profile_hook = lambda: holder[0]
        sys.modules["antenv.axon_hooks"] = mod
        antenv.axon_hooks = mod
        mod.set_axon_ntff_profile_hook(
            _ntff_profile_via_ctypes("/opt/axon/libaxon_pjrt.so"))
        return True
    except Exception:
        return False


def _build(C):
    import concourse.mybir as mybir
    import concourse.tile as tile
    from concourse import bacc

    nc = bacc.Bacc(None, target_bir_lowering=False)

    slices = _chunks(C)
    # One contiguous DRAM param per n-slice: the HW queue ring delivers them
    # in order, so the first GEMM group can start on x0 while x1/x2 stream.
    xs = [nc.declare_dram_parameter(f"x{i}", [P, KD, sl.stop - sl.start],
                                    mybir.dt.bfloat16, isOutput=False)
          for i, sl in enumerate(slices)]
    w1 = nc.declare_dram_parameter("w1", [MH, P, KD, P], mybir.dt.bfloat16, isOutput=False)
    b1 = nc.declare_dram_parameter("b1", [P, MH], mybir.dt.float32, isOutput=False)
    w2 = nc.declare_dram_parameter("w2", [KD, P, MH, P], mybir.dt.bfloat16, isOutput=False)
    b2 = nc.declare_dram_parameter("b2", [P, KD], mybir.dt.float32, isOutput=False)
    out = nc.declare_dram_parameter("out", [P, KD, C], mybir.dt.float32, isOutput=True)

    GELU = mybir.ActivationFunctionType.Gelu

    with tile.TileContext(nc) as tc, \
         tc.tile_pool(name="singles", bufs=1) as singles, \
         tc.tile_pool(name="w1pool", bufs=3) as w1pool, \
         tc.tile_pool(name="w2pool", bufs=3) as w2pool, \
         tc.tile_pool(name="ypool", bufs=3) as ypool, \
         tc.tile_pool(name="psum", bufs=4, space="PSUM") as psum_pool:

        # PE warm-up: dependency-free matmuls keep the TensorEngine busy
        # through the ~10us DMA ring-init + xT/w1 transfer window, so the HAM
        # clock gate is released (2.4 GHz) and the PE is warm when the real
        # stream starts (~17us).  ~16 run cold (3.4us), the rest at ~107ns.
        warm_sb = singles.tile([P, 2 * P], mybir.dt.bfloat16)
        nc.vector.memset(warm_sb[:], 0.0)
        ps_warm = psum_pool.tile([P, 2 * P], mybir.dt.float32, name="ps_warm",
                                 tag="ps1")
        for _ in range(28):
            nc.tensor.matmul(ps_warm[:], warm_sb[:, :P], warm_sb[:],
                             start=True, stop=True)

        # DMA issue order: the HW queue ring drains in issue order, so the
        # first GEMM group's deps (w1 tile 0, b1, x slice 0) go first.
        w1_t0 = w1pool.tile([P, KD, P], mybir.dt.bfloat16, name="w1_t")
        nc.sync.dma_start(out=w1_t0[:], in_=w1[0])
        b1_sb = singles.tile([P, MH], mybir.dt.float32)
        nc.sync.dma_start(out=b1_sb[:], in_=b1[:])
        x_sbs = []
        for i, sl in enumerate(slices):
            x_sb = singles.tile([P, KD, sl.stop - sl.start], mybir.dt.bfloat16,
                                name=f"x_sb{i}")
            nc.sync.dma_start(out=x_sb[:], in_=xs[i][:])
            x_sbs.append(x_sb)
        b2_sb = singles.tile([P, KD], mybir.dt.float32)
        nc.sync.dma_start(out=b2_sb[:], in_=b2[:])
        hT_sb = singles.tile([P, MH, C], mybir.dt.bfloat16)

        # GEMM1: hT[mo*128+p, c] = gelu(sum_k w1[k,:].T @ xT[k,:] + b1)
        for mo in range(MH):
            if mo == 0:
                w1_t = w1_t0
            else:
                w1_t = w1pool.tile([P, KD, P], mybir.dt.bfloat16, name="w1_t")
                nc.sync.dma_start(out=w1_t[:], in_=w1[mo])
            for si, sl in enumerate(slices):
                ps1 = psum_pool.tile([P, sl.stop - sl.start], mybir.dt.float32,
                                     name="ps1")
                for k in range(KD):
                    nc.tensor.matmul(ps1[:], w1_t[:, k, :], x_sbs[si][:, k, :],
                                     start=(k == 0), stop=(k == KD - 1))
                nc.scalar.activation(hT_sb[:, mo, sl], ps1[:], GELU,
                                     bias=b1_sb[:, mo:mo + 1])

        # GEMM2: yT[do*128+p, c] = sum_k w2[k,:].T @ hT[k,:] + b2
        for do in range(KD):
            w2_t = w2pool.tile([P, MH, P], mybir.dt.bfloat16, name="w2_t")
            nc.sync.dma_start(out=w2_t[:], in_=w2[do])
            for sl in slices:
                ps2 = psum_pool.tile([P, sl.stop - sl.start], mybir.dt.float32,
                                     name="ps2")
                for k in range(MH):
                    nc.tensor.matmul(ps2[:], w2_t[:, k, :], hT_sb[:, k, sl],
                                     start=(k == 0), stop=(k == MH - 1))
                y_sb = ypool.tile([P, sl.stop - sl.start], mybir.dt.float32,
                                  name="y_sb")
                nc.vector.tensor_scalar_add(y_sb[:], ps2[:], b2_sb[:, do:do + 1])
                nc.sync.dma_start(out=out[:, do, sl], in_=y_sb[:])

    nc.compile()
    return nc


def kernel(x, gate_w, gate_b, w1, b1, w2, b2):
    global LAST_EXEC_TIME_NS
    from concourse.bass_utils import run_bass_kernel_spmd

    x = np.asarray(x)
    xf = np.ascontiguousarray(x.reshape(N, D), dtype=np.float32)

    # --- Gate (host, float64 for a stable top-2 selection) ---
    logits = xf.astype(np.float64) @ np.asarray(gate_w).astype(np.float64)
    logits += np.asarray(gate_b).astype(np.float64)
    rows = np.arange(N)
    i1 = np.argmax(logits, axis=1)
    l1 = logits[rows, i1]
    tmp = logits.copy()
    tmp[rows, i1] = -np.inf
    i2 = np.argmax(tmp, axis=1)
    l2 = tmp[rows, i2]
    e2 = np.exp(l2 - l1)          # l1 >= l2
    wa = (1.0 / (1.0 + e2)).astype(np.float32)
    wb = (e2 / (1.0 + e2)).astype(np.float32)

    # --- Dispatch (host): per-expert token lists; C = max load (runtime) ---
    sels, wgts = [], []
    for e in range(E):
        sel = np.where((i1 == e) | (i2 == e))[0]
        wgt = np.where(i1[sel] == e, wa[sel], wb[sel])
        sels.append(sel)
        wgts.append(wgt)
    C = max(256, max(len(s) for s in sels))

    # --- Per-core input maps ---
    w1a = np.asarray(w1, dtype=np.float32)
    b1a = np.asarray(b1, dtype=np.float32)
    w2a = np.asarray(w2, dtype=np.float32)
    b2a = np.asarray(b2, dtype=np.float32)
    cslices = _chunks(C)
    in_maps = []
    for e in range(E):
        xe = np.zeros((C, D), dtype=np.float32)
        xe[:len(sels[e])] = xf[sels[e]]
        xT_r = xe.T.reshape(KD, P, C).transpose(1, 0, 2).astype(BF16)
        w1_r = np.ascontiguousarray(
            w1a[e].reshape(KD, P, MH, P).transpose(2, 1, 0, 3)).astype(BF16)
        w2_r = np.ascontiguousarray(
            w2a[e].reshape(MH, P, KD, P).transpose(2, 1, 0, 3)).astype(BF16)
        b1_r = np.ascontiguousarray(b1a[e].reshape(MH, P).T)
        b2_r = np.ascontiguousarray(b2a[e].reshape(KD, P).T)
        m = {"w1": w1_r, "b1": b1_r, "w2": w2_r, "b2": b2_r}
        for i, sl in enumerate(cslices):
            m[f"x{i}"] = np.ascontiguousarray(xT_r[:, :, sl])
        in_maps.append(m)

    if C not in _cached_nc:
        _cached_nc[C] = _build(C)
    nc = _cached_nc[C]

    trace = os.environ.get("MOE_KERNEL_PROFILE", "0") == "1"
    if trace:
        trace = _ensure_ntff_hook()
    res = None
    for attempt in range(3):
        try:
            res = run_bass_kernel_spmd(nc, in_maps, core_ids=list(range(E)),
                                       trace=trace and attempt == 0)
            break
        except Exception:
            # Device-unrecoverable NRT errors are transient here; retry with
            # a fresh PJRT client (last attempt re-raises).
            if attempt == 2:
                raise
            try:
                import jax
                jax.clear_caches()
                jax._src.api.clear_backends()
            except Exception:
                pass
    LAST_EXEC_TIME_NS = res.exec_time_ns

    # --- Combine (host) ---
    out_acc = np.zeros((N, D), dtype=np.float32)
    for e in range(E):
        yT = np.asarray(res.results[e]["out"])          # [P, KD, C] f32
        y = yT.transpose(1, 0, 2).reshape(D, C).T       # [C, D]
        ne = len(sels[e])
        out_acc[sels[e]] += wgts[e][:, None] * y[:ne]

    return out_acc.reshape(B, T, D)



# revision 4
# speedup vs baseline: 1.0500x; 1.0500x over previous
"""MoE layer (top-2 of 8 experts) on 8 Trainium2 NeuronCores.

Strategy (self-contained; shapes hardcoded for B=4,T=1024,D=1024,E=8,K=2,H=4096):
  - Host: gate logits + top-2 + softmax (float64 for a stable selection).
  - Slot-cover dispatch: every core runs the SAME kernel with two
    fixed-capacity column segments (s1 >= s2).  Chip-wide that gives 8 slots
    of cap s1 and 8 of cap s2; a small DP assigns each expert a set of slots
    (e.g. a 1101-token expert takes two s1 slots on different cores, a
    1035-token expert takes one s1 + one s2).  This shrinks the per-core
    column count C = s1+s2 from max_e(load_e) (~1101) toward the perfect
    balance sum/8 = 1024 while keeping one compile-time kernel structure.
  - Device, SPMD over 8 cores: per segment a transposed-layout FFN
      hT = gelu(w1.T @ xT + b1)   [H, s]
      yT = w2.T @ hT + b2         [D, s]
    bf16 matmul inputs, f32 PSUM accumulation; outputs DMA'd as bf16.
    Weight tiles are stationary (lhsT); tokens stream as the moving operand,
    so the column capacity per segment is exact (no 128-quantization).
    All chunks are >=256 cols so LDWEIGHTS (~100ns) hides under the stream.
  - Host combine: weighted scatter-add of the slot outputs.
"""

import os

import numpy as np
import ml_dtypes

B, T, D = 4, 1024, 1024
E, K, H = 8, 2, 4 * 1024
N = B * T
P = 128
KD = D // P           # 8  k-tiles in GEMM1 / output d-tiles in GEMM2
MH = H // P           # 32 m-tiles in GEMM1 / k-tiles in GEMM2
BF16 = ml_dtypes.bfloat16
WARMUP_MM = 24

LAST_EXEC_TIME_NS = None
_cached_nc = {}


def _chunks(c):
    """Split a segment's columns into PSUM-bank-sized (<=512 f32) slices.

    Near-equal so every chunk stays >=256 for any c>=256, keeping LDWEIGHTS
    hidden under the matmul stream.
    """
    n = -(-c // 512)
    base = c // n
    rem = c - base * n
    sizes = [base + (1 if i < rem else 0) for i in range(n)]
    out, off = [], 0
    for s in sizes:
        out.append(slice(off, off + s))
        off += s
    return out


def _solve_slots(loads):
    """Pick segment caps (s1, s2) and an expert->slot assignment.

    8 slots of cap s1 and 8 of cap s2 (one of each per core).  Expert e
    claims j s1-slots + k s2-slots with j*s1 + k*s2 >= loads[e].
    Minimizes C = s1 + s2; among ties prefers the largest minimum chunk.
    Returns (s1, s2, combos) with combos[e] = (j, k).
    """
    loads = list(map(int, loads))
    maxload = max(loads)
    lb = max(512, -(-sum(loads) // 8))
    c0 = 2 * (-(-maxload // 2))          # always-feasible fallback: (1,1) each

    def feasible(s1, s2):
        # Pareto-minimal combos per expert, then DP over (used1, used2).
        combo_opts = []
        for n_e in loads:
            opts = []
            if n_e == 0:
                opts.append((0, 0))
            else:
                for j in range(0, 9):
                    rest = n_e - j * s1
                    k = 0 if rest <= 0 else -(-rest // s2)
                    if k <= 8:
                        opts.append((j, k))
                        if rest <= 0:
                            break
            if not opts:
                return None
            combo_opts.append(opts)
        states = {(0, 0): []}
        for opts in combo_opts:
            nxt = {}
            for (u1, u2), hist in states.items():
                for (j, k) in opts:
                    v1, v2 = u1 + j, u2 + k
                    if v1 <= 8 and v2 <= 8 and (v1, v2) not in nxt:
                        nxt[(v1, v2)] = hist + [(j, k)]
            if not nxt:
                return None
            states = nxt
        return next(iter(states.values()))

    for C in range(lb, c0 + 1):
        best = None
        for s1 in range(-(-C // 2), C - 256 + 1):
            s2 = C - s1
            if s2 < 256:
                break
            combos = feasible(s1, s2)
            if combos is None:
                continue
            minchunk = min(min(sl.stop - sl.start for sl in _chunks(s))
                           for s in (s1, s2))
            if best is None or minchunk > best[0]:
                best = (minchunk, s1, s2, combos)
        if best is not None:
            return best[1], best[2], best[3]
    # unreachable (c0 config is feasible), but keep a hard fallback
    s = -(-maxload // 2)
    return s, s, [(1, 1) for _ in loads]


def _ensure_ntff_hook():
    """Register the axon NTFF profile hook if the image lacks antenv.axon_hooks."""
    import sys
    import types
    try:
        from antenv.axon_hooks import get_axon_ntff_profile_hook
        return get_axon_ntff_profile_hook() is not None
    except ImportError:
        pass
    try:
        import antenv
        from trn_agent_boot.trn_boot import _ntff_profile_via_ctypes
        mod = types.ModuleType("antenv.axon_hooks")
        holder = [None]
        mod.set_axon_ntff_profile_hook = lambda h: holder.__setitem__(0, h)
        mod.get_axon_ntff_profile_hook = lambda: holder[0]
        sys.modules["antenv.axon_hooks"] = mod
        antenv.axon_hooks = mod
        mod.set_axon_ntff_profile_hook(
            _ntff_profile_via_ctypes("/opt/axon/libaxon_pjrt.so"))
        return True
    except Exception:
        return False


def _build(s1, s2):
    import concourse.mybir as mybir
    import concourse.tile as tile
    from concourse import bacc

    nc = bacc.Bacc(None, target_bir_lowering=False)

    segs = [("A", s1), ("B", s2)]
    xs = {}
    w1p = {}
    b1p = {}
    w2p = {}
    b2p = {}
    outp = {}
    for sname, cap in segs:
        xs[sname] = [nc.declare_dram_parameter(
            f"x{sname}{i}", [P, KD, sl.stop - sl.start],
            mybir.dt.bfloat16, isOutput=False)
            for i, sl in enumerate(_chunks(cap))]
        w1p[sname] = nc.declare_dram_parameter(
            f"w1{sname}", [MH, P, KD, P], mybir.dt.bfloat16, isOutput=False)
        b1p[sname] = nc.declare_dram_parameter(
            f"b1{sname}", [P, MH], mybir.dt.float32, isOutput=False)
        w2p[sname] = nc.declare_dram_parameter(
            f"w2{sname}", [KD, P, MH, P], mybir.dt.bfloat16, isOutput=False)
        b2p[sname] = nc.declare_dram_parameter(
            f"b2{sname}", [P, KD], mybir.dt.float32, isOutput=False)
        outp[sname] = nc.declare_dram_parameter(
            f"out{sname}", [P, KD, cap], mybir.dt.bfloat16, isOutput=True)

    GELU = mybir.ActivationFunctionType.Gelu

    with tile.TileContext(nc) as tc, \
         tc.tile_pool(name="singles", bufs=1) as singles, \
         tc.tile_pool(name="w1poolA", bufs=3) as w1poolA, \
         tc.tile_pool(name="w1poolB", bufs=3) as w1poolB, \
         tc.tile_pool(name="w2poolA", bufs=2) as w2poolA, \
         tc.tile_pool(name="w2poolB", bufs=2) as w2poolB, \
         tc.tile_pool(name="ypool", bufs=3) as ypool, \
         tc.tile_pool(name="psum", bufs=4, space="PSUM") as psum_pool:

        w1pool = {"A": w1poolA, "B": w1poolB}
        w2pool = {"A": w2poolA, "B": w2poolB}

        # PE warm-up: dependency-free matmuls keep the TensorEngine busy
        # through the DMA ring-init + first-data window and release the HAM
        # clock gate, so the PE is warm when the real stream starts.
        warm_sb = singles.tile([P, 2 * P], mybir.dt.bfloat16)
        nc.vector.memset(warm_sb[:], 0.0)
        ps_warm = psum_pool.tile([P, 2 * P], mybir.dt.float32, name="ps_warm",
                                 tag="ps1")
        for _ in range(WARMUP_MM):
            nc.tensor.matmul(ps_warm[:], warm_sb[:, :P], warm_sb[:],
                             start=True, stop=True)

        # DMA issue order: the queue rings drain roughly in issue order, so
        # the first GEMM group's deps (w1A tile 0, xA chunk 0) go first.
        w1_t0 = {}
        x_sbs = {}
        b1_sb = {}
        b2_sb = {}
        h_sb = {}
        for sname, cap in segs:
            w1_t0[sname] = w1pool[sname].tile([P, KD, P], mybir.dt.bfloat16,
                                              name=f"w1{sname}_t")
            nc.sync.dma_start(out=w1_t0[sname][:], in_=w1p[sname][0])
            b1_sb[sname] = singles.tile([P, MH], mybir.dt.float32,
                                         name=f"b1{sname}_sb")
            nc.sync.dma_start(out=b1_sb[sname][:], in_=b1p[sname][:])
            x_sbs[sname] = []
            for i, sl in enumerate(_chunks(cap)):
                x_sb = singles.tile([P, KD, sl.stop - sl.start],
                                    mybir.dt.bfloat16, name=f"x{sname}{i}")
                nc.sync.dma_start(out=x_sb[:], in_=xs[sname][i][:])
                x_sbs[sname].append(x_sb)
        for sname, cap in segs:
            b2_sb[sname] = singles.tile([P, KD], mybir.dt.float32,
                                         name=f"b2{sname}_sb")
            nc.sync.dma_start(out=b2_sb[sname][:], in_=b2p[sname][:])
            h_sb[sname] = singles.tile([P, MH, cap], mybir.dt.bfloat16,
                                        name=f"h{sname}_sb")

        # GEMM1: hT[mo*128+p, c] = gelu(sum_k w1[k,:].T @ xT[k,:] + b1)
        for mo in range(MH):
            for sname, cap in segs:
                if mo == 0:
                    w1_t = w1_t0[sname]
                else:
                    w1_t = w1pool[sname].tile([P, KD, P], mybir.dt.bfloat16,
                                              name=f"w1{sname}_t")
                    nc.sync.dma_start(out=w1_t[:], in_=w1p[sname][mo])
                for si, sl in enumerate(_chunks(cap)):
                    ps1 = psum_pool.tile([P, sl.stop - sl.start],
                                         mybir.dt.float32, name="ps1")
                    for k in range(KD):
                        nc.tensor.matmul(ps1[:], w1_t[:, k, :],
                                         x_sbs[sname][si][:, k, :],
                                         start=(k == 0), stop=(k == KD - 1))
                    nc.scalar.activation(h_sb[sname][:, mo, sl], ps1[:], GELU,
                                         bias=b1_sb[sname][:, mo:mo + 1])

        # GEMM2: yT[do*128+p, c] = sum_k w2[k,:].T @ hT[k,:] + b2
        for do in range(KD):
            for sname, cap in segs:
                w2_t = w2pool[sname].tile([P, MH, P], mybir.dt.bfloat16,
                                          name=f"w2{sname}_t")
                nc.sync.dma_start(out=w2_t[:], in_=w2p[sname][do])
                for sl in _chunks(cap):
                    ps2 = psum_pool.tile([P, sl.stop - sl.start],
                                         mybir.dt.float32, name="ps2")
                    for k in range(MH):
                        nc.tensor.matmul(ps2[:], w2_t[:, k, :],
                                         h_sb[sname][:, k, sl],
                                         start=(k == 0), stop=(k == MH - 1))
                    y_sb = ypool.tile([P, sl.stop - sl.start],
                                      mybir.dt.bfloat16, name="y_sb")
                    nc.vector.tensor_scalar_add(y_sb[:], ps2[:],
                                                b2_sb[sname][:, do:do + 1])
                    nc.sync.dma_start(out=outp[sname][:, do, sl], in_=y_sb[:])

    nc.compile()
    return nc


def kernel(x, gate_w, gate_b, w1, b1, w2, b2):
    global LAST_EXEC_TIME_NS
    from concourse.bass_utils import run_bass_kernel_spmd

    x = np.asarray(x)
    xf = np.ascontiguousarray(x.reshape(N, D), dtype=np.float32)

    # --- Gate (host, float64 for a stable top-2 selection) ---
    logits = xf.astype(np.float64) @ np.asarray(gate_w).astype(np.float64)
    logits += np.asarray(gate_b).astype(np.float64)
    rows = np.arange(N)
    i1 = np.argmax(logits, axis=1)
    l1 = logits[rows, i1]
    tmp = logits.copy()
    tmp[rows, i1] = -np.inf
    i2 = np.argmax(tmp, axis=1)
    l2 = tmp[rows, i2]
    e2 = np.exp(l2 - l1)          # l1 >= l2
    wa = (1.0 / (1.0 + e2)).astype(np.float32)
    wb = (e2 / (1.0 + e2)).astype(np.float32)

    # --- Dispatch: per-expert token lists ---
    sels, wgts = [], []
    for e in range(E):
        sel = np.where((i1 == e) | (i2 == e))[0]
        wgt = np.where(i1[sel] == e, wa[sel], wb[sel])
        sels.append(sel)
        wgts.append(wgt)
    loads = [len(s) for s in sels]

    # --- Slot cover: segment caps + expert->slot assignment ---
    s1, s2, combos = _solve_slots(loads)
    caps = {"A": s1, "B": s2}

    # slot_expert[seg][core] = expert owning that slot (or -1 = unused)
    # slot_fill[seg][core] = (expert, start_idx, count) token range
    slot_expert = {"A": [-1] * E, "B": [-1] * E}
    slot_fill = {"A": [None] * E, "B": [None] * E}
    next_slot = {"A": 0, "B": 0}
    for e in range(E):
        j, k = combos[e]
        claimed = [("A", next_slot["A"] + i) for i in range(j)] + \
                  [("B", next_slot["B"] + i) for i in range(k)]
        next_slot["A"] += j
        next_slot["B"] += k
        off = 0
        for sname, slot in claimed:
            cnt = min(caps[sname], loads[e] - off)
            slot_expert[sname][slot] = e
            slot_fill[sname][slot] = (e, off, max(cnt, 0))
            off += max(cnt, 0)
        assert off >= loads[e], f"slot cover failed for expert {e}"

    # --- Per-core input maps ---
    w1a = np.asarray(w1, dtype=np.float32)
    b1a = np.asarray(b1, dtype=np.float32)
    w2a = np.asarray(w2, dtype=np.float32)
    b2a = np.asarray(b2, dtype=np.float32)
    wcache = {}

    def expert_weights(e):
        if e not in wcache:
            w1_r = np.ascontiguousarray(
                w1a[e].reshape(KD, P, MH, P).transpose(2, 1, 0, 3)).astype(BF16)
            w2_r = np.ascontiguousarray(
                w2a[e].reshape(MH, P, KD, P).transpose(2, 1, 0, 3)).astype(BF16)
            b1_r = np.ascontiguousarray(b1a[e].reshape(MH, P).T)
            b2_r = np.ascontiguousarray(b2a[e].reshape(KD, P).T)
            wcache[e] = (w1_r, b1_r, w2_r, b2_r)
        return wcache[e]

    in_maps = []
    for c in range(E):
        m = {}
        for sname in ("A", "B"):
            cap = caps[sname]
            fill = slot_fill[sname][c]
            xe = np.zeros((cap, D), dtype=np.float32)
            eid = 0
            if fill is not None:
                eid, off, cnt = fill
                xe[:cnt] = xf[sels[eid][off:off + cnt]]
            xT_r = xe.T.reshape(KD, P, cap).transpose(1, 0, 2).astype(BF16)
            w1_r, b1_r, w2_r, b2_r = expert_weights(eid)
            m[f"w1{sname}"] = w1_r
            m[f"b1{sname}"] = b1_r
            m[f"w2{sname}"] = w2_r
            m[f"b2{sname}"] = b2_r
            for i, sl in enumerate(_chunks(cap)):
                m[f"x{sname}{i}"] = np.ascontiguousarray(xT_r[:, :, sl])
        in_maps.append(m)

    key = (s1, s2)
    if key not in _cached_nc:
        _cached_nc[key] = _build(s1, s2)
    nc = _cached_nc[key]

    trace = os.environ.get("MOE_KERNEL_PROFILE", "0") == "1"
    if trace:
        trace = _ensure_ntff_hook()
    res = None
    for attempt in range(3):
        try:
            res = run_bass_kernel_spmd(nc, in_maps, core_ids=list(range(E)),
                                       trace=trace and attempt == 0)
            break
        except Exception:
            # Device-unrecoverable NRT errors are transient here; retry with
            # a fresh PJRT client (last attempt re-raises).
            if attempt == 2:
                raise
            try:
                import jax
                jax.clear_caches()
                jax._src.api.clear_backends()
            except Exception:
                pass
    LAST_EXEC_TIME_NS = res.exec_time_ns

    # --- Combine (host) ---
    out_acc = np.zeros((N, D), dtype=np.float32)
    for sname in ("A", "B"):
        cap = caps[sname]
        for c in range(E):
            fill = slot_fill[sname][c]
            if fill is None or fill[2] == 0:
                continue
            e, off, cnt = fill
            yT = np.asarray(res.results[c][f"out{sname}"]).astype(np.float32)
            y = yT.transpose(1, 0, 2).reshape(D, cap).T    # [cap, D]
            toks = sels[e][off:off + cnt]
            out_acc[toks] += wgts[e][off:off + cnt, None] * y[:cnt]

    return out_acc.reshape(B, T, D)
